# revision 2
# baseline (speedup 1.0000x reference)
"""Equivariant rotation conv for Trainium2, 8-core batch-parallel,
F(2,3) Winograd along H.

Computes: rotate a (128*8, 128, 3, 3) filter bank by 8 data-dependent angles
(bilinear resampling), run a 3x3 same-padded conv of x (16,128,128,128) with
all 8*128 rotated filters, then max over the 8 rotations -> (16,128,128,128).

Sharding: data-parallel over batch, 2 images per core; weights replicated.

Algorithm (per core):
  - Weight prep happens fully on host: the 9x9 bilinear rotation-mixing
    matrix is applied to each rotation's filter bank, then the Winograd
    F(2,3) G-transform is folded along ky, giving per rotation 4 H-taps x
    3 kx-taps of [cin, O] bf16 lhsT matrices (shipped, no device prep).
  - Input transform on DVE: V_t[j] over row pairs (4 taps, 1 add each,
    f32 in -> bf16 out), columns padded same as the direct kernel.
  - Per 8-output-row "pair" per rotation: 24 matmuls (4 t x 3 kx x 2
    half-groups of N=512) accumulate kx in f32 PSUM (2x 4-bank tiles,
    alternating halves).
  - ScalarE evacuates each 4-bank PSUM half (2048 f32) to SBUF bf16 --
    this keeps the DVE in its 2x bf16 mode and off the slow PSUM port.
  - DVE inverse transform: yev = (m0+m1)+m2, yod = (m1-m2)-m3, then a
    running elementwise max over rotations (skipped for r=0; the final
    rotation fuses max with the f32 upcast via scalar_tensor_tensor).
  - PE work is 2/3 of the direct 9-tap conv: ~164 matmul-cycles per
    output pixel per rotation instead of 288.
"""

import numpy as np


def _install_axon_hooks_shim():
    """Provide antenv.axon_hooks (NTFF profile hook) when the image's antenv
    lacks it, so run_bass_kernel_spmd(trace=True) works instead of crashing
    on import."""
    import contextlib
    import ctypes
    import os
    import sys
    import types

    try:
        import antenv.axon_hooks  # noqa: F401

        return
    except ImportError:
        pass

    state = {"hook": None, "resolved": False}

    def _make_hook():
        so_path = os.environ.get("AXON_PJRT_SO", "/opt/axon/libaxon_pjrt.so")
        if not os.path.exists(so_path):
            return None
        lib = ctypes.CDLL(so_path)
        if not hasattr(lib, "axon_start_nrt_profile"):
            return None
        lib.axon_start_nrt_profile.argtypes = [
            ctypes.POINTER(ctypes.c_int64),
            ctypes.c_size_t,
        ]
        lib.axon_start_nrt_profile.restype = ctypes.c_int64
        lib.axon_stop_nrt_profile.argtypes = [ctypes.c_char_p]
        lib.axon_stop_nrt_profile.restype = ctypes.c_int64

        @contextlib.contextmanager
        def _hook(output_dir, device_ids):
            import jax

            jax.devices()
            if device_ids:
                ids = (ctypes.c_int64 * len(device_ids))(*device_ids)
                rc = lib.axon_start_nrt_profile(ids, len(device_ids))
            else:
                rc = lib.axon_start_nrt_profile(None, 0)
            if rc != 0:
                raise RuntimeError(f"axon_start_nrt_profile rc={rc}")
            try:
                yield
            finally:
                n = lib.axon_stop_nrt_profile(str(output_dir).encode())
                if n < 0:
                    raise RuntimeError(f"axon_stop_nrt_profile rc={n}")
                print(f"profile: {n} file(s) written to {output_dir}")

        return _hook

    mod = types.ModuleType("antenv.axon_hooks")

    def set_axon_ntff_profile_hook(h):
        state["hook"] = h
        state["resolved"] = True

    def get_axon_ntff_profile_hook():
        if not state["resolved"]:
            state["hook"] = _make_hook()
            state["resolved"] = True
        return state["hook"]

    mod.set_axon_ntff_profile_hook = set_axon_ntff_profile_hook
    mod.get_axon_ntff_profile_hook = get_axon_ntff_profile_hook
    sys.modules["antenv.axon_hooks"] = mod


_install_axon_hooks_shim()

import ml_dtypes

import concourse.bass as bass
import concourse.mybir as mybir
from concourse import bacc
from concourse.bass_utils import run_bass_kernel_spmd
from concourse.tile import TileContext

F32 = mybir.dt.float32
BF16 = mybir.dt.bfloat16
ALU = mybir.AluOpType

B, CIN, H, W = 16, 128, 128, 128
R, O, K = 8, 128, 3
NCORES = 8
BL = B // NCORES   # images per core
RB = 32            # output rows per block
JB = RB // 2       # winograd row-pairs per block
NBLK = H // RB

_TRACE = False
LAST_RESULTS = None
_NC_CACHE = {}


def _rot_mats(rot_alpha):
    """Per-rotation 9x9 bilinear resampling matrices, matching the reference
    F.grid_sample(align_corners=True, zeros) tap logic exactly."""
    M = np.zeros((R, 9, 9), np.float64)
    lin = np.linspace(-1.0, 1.0, K)
    for r in range(R):
        ang = float(rot_alpha[r]) * (np.pi / 4.0) * r
        c, s = np.cos(ang), np.sin(ang)
        for a in range(K):          # output row (gy = lin[a])
            for b in range(K):      # output col (gx = lin[b])
                gx, gy = lin[b], lin[a]
                xs = c * gx - s * gy
                ys = s * gx + c * gy
                ix = (xs + 1.0) * 0.5 * (K - 1)
                iy = (ys + 1.0) * 0.5 * (K - 1)
                x0 = int(np.floor(ix))
                y0 = int(np.floor(iy))
                wx, wy = ix - x0, iy - y0
                p = a * K + b
                for yi, xi, wt in (
                    (y0, x0, (1 - wy) * (1 - wx)),
                    (y0, x0 + 1, (1 - wy) * wx),
                    (y0 + 1, x0, wy * (1 - wx)),
                    (y0 + 1, x0 + 1, wy * wx),
                ):
                    if 0 <= yi < K and 0 <= xi < K:
                        M[r, p, yi * K + xi] += wt
    return M


def _wino_weights(weight, rot_alpha):
    """lhsT bank [cin, R, 12, O] bf16: rotated filters with the F(2,3)
    G-transform folded along ky; 12 = (t, kx) pairs."""
    M = _rot_mats(rot_alpha)
    Bw = weight.reshape(O, R, CIN, K * K).astype(np.float64)  # [o, r, i, q]
    wr = np.einsum("rpq,oriq->roip", M, Bw).reshape(R, O, CIN, K, K)
    G = np.array(
        [[1, 0, 0], [0.5, 0.5, 0.5], [0.5, -0.5, 0.5], [0, 0, 1]], np.float64
    )
    wt = np.einsum("ty,roiyx->irtxo", G, wr)  # [i, r, t, kx, o]
    wt = wt.reshape(CIN, R, 12, O).astype(np.float32)
    return np.ascontiguousarray(wt).astype(ml_dtypes.bfloat16)


def _build():
    nc = bacc.Bacc(trn_type="TRN2")
    xs = nc.dram_tensor("xs", [BL, CIN, H, W], F32, kind="ExternalInput")
    wl = nc.dram_tensor("wl", [CIN, R, 12, O], BF16, kind="ExternalInput")
    y = nc.dram_tensor("y", [BL, O, H, W], F32, kind="ExternalOutput")

    with TileContext(nc) as tc:
        with (
            tc.tile_pool(name="wgt", bufs=1) as wpool,
            tc.tile_pool(name="xio", bufs=1) as xpool,
            tc.tile_pool(name="vst", bufs=2) as vpool,
            tc.tile_pool(name="mst", bufs=5) as mpool,
            tc.tile_pool(name="yst", bufs=2) as ypool,
            tc.tile_pool(name="accp", bufs=2) as apool,
            tc.tile_pool(name="outp", bufs=2) as opool,
            tc.tile_pool(name="psum", bufs=1, space="PSUM") as ppool,
        ):
            wt = wpool.tile([128, R, 12, O], BF16, name="wt", tag="wt")

            # PE warm-up: dependency-free matmuls keep the PE busy from
            # ~0.5us so the HAM clock gate reaches 8/8 before real work
            # starts at ~6us.
            # dum_lhs zeroing must be the vector engine's first op: ACT pays
            # a ~2.7us activation-table load before its first instruction,
            # and gpsimd a ~6us ucode load -- either would delay the warm-up.
            dum_lhs = wpool.tile([128, 128], BF16, name="dum_lhs", tag="dum")
            nc.vector.memset(dum_lhs[:, :], 0.0)
            # r=0 weights issued from the scalar engine's DMA queue (its
            # first instruction): transfers in parallel with block 0's first
            # x rows on the sync queue
            nc.scalar.dma_start(out=wt[:, 0], in_=wl[:, 0])
            dum_ps = ppool.tile([128, 4, 4, 128], F32, name="dum_ps", tag="psA")
            for _ in range(44):
                nc.tensor.matmul(
                    dum_ps[:, 0, 0, :], dum_lhs[:, :], dum_lhs[:, :],
                    start=True, stop=True,
                )

            # x staging: manual ping-pong between two persistent buffers so
            # the zero padding (columns 0 and W+1, boundary halo rows) is
            # established once instead of re-memset every block.
            # Only the pad regions need zeroing (DMA always writes cols
            # 1..W, so cols 0 and W+1 are established once; halo rows are
            # re-zeroed by load_x when a buffer is reused at an image edge).
            # Vector-engine memsets (fast, and the scalar queue must stay
            # clear so the r=0 weights DMA issues immediately).
            xst2 = [
                xpool.tile([128, RB + 2, W + 2], F32, name=f"xst{i}", tag=f"xst{i}")
                for i in range(2)
            ]
            for i in range(2):
                nc.vector.memset(xst2[i][:, :, 0:1], 0.0)
                nc.vector.memset(xst2[i][:, :, W + 1 : W + 2], 0.0)
                nc.vector.memset(xst2[i][:, 0:1, 1 : W + 1], 0.0)
                nc.vector.memset(xst2[i][:, RB + 1 : RB + 2, 1 : W + 1], 0.0)

            def load_x(g, b, blk, cuts=None):
                h0 = blk * RB
                r0 = max(h0 - 1, 0)
                r1 = min(h0 + RB + 1, H)
                xst = xst2[g % 2]
                if g >= 2:
                    # restore halo-row zeros clobbered by the previous user
                    # of this buffer (interior blocks write all 34 rows)
                    if blk == 0:
                        nc.gpsimd.memset(xst[:, 0:1, :], 0.0)
                    elif blk == NBLK - 1:
                        nc.gpsimd.memset(xst[:, RB + 1 : RB + 2, :], 0.0)
                d0 = r0 - (h0 - 1)
                nrows = r1 - r0
                if cuts is None:
                    cuts = [0, nrows]
                for k in range(len(cuts) - 1):
                    a, c = cuts[k], cuts[k + 1]
                    nc.sync.dma_start(
                        out=xst[:, d0 + a : d0 + c, 1 : W + 1],
                        in_=xs[b, :, r0 + a : r0 + c, :],
                    )
                return xst

            # V input transform: V0=x[2j-1]-x[2j+1], V1=x[2j]+x[2j+1],
            # V2=x[2j+1]-x[2j], V3=x[2j]-x[2j+3+...]; xst row d holds
            # x row (h0-1)+d so x[2j+k] is xst row 2jl+k+1.
            VSPEC = [
                (0, 2, ALU.subtract),
                (1, 2, ALU.add),
                (2, 1, ALU.subtract),
                (1, 3, ALU.subtract),
            ]

            def emit_v(xst, v, jcuts=(0, JB)):
                for k in range(len(jcuts) - 1):
                    j0, j1 = jcuts[k], jcuts[k + 1]
                    for t, (a, bb, op) in enumerate(VSPEC):
                        nc.vector.tensor_tensor(
                            v[:, t, j0:j1, :],
                            xst[:, a + 2 * j0 : a + 2 * j1 - 1 : 2, :],
                            xst[:, bb + 2 * j0 : bb + 2 * j1 - 1 : 2, :],
                            op,
                        )

            def stage(g, cuts=None, jcuts=(0, JB)):
                b, blk = divmod(g, NBLK)
                xst = load_x(g, b, blk, cuts=cuts)
                v = vpool.tile([128, 4, JB, W + 2], BF16, name=f"v{g}", tag="v")
                emit_v(xst, v, jcuts)
                return v

            def conv_pair(v, acc, obuf, r, p, store=None, split_final=False):
                # One pair = 8 output rows (j = 8p .. 8p+8).  24 matmuls in
                # two 4-bank halves; ScalarE evacuates each half to bf16;
                # DVE does the 4-add inverse + running max.  split_final
                # (the very last pair) pipelines everything per half so the
                # post-matmul tail is short.
                ms = mpool.tile([128, 4, 8, 128], BF16, name="ms", tag="ms")
                for h in range(2):
                    pst = ppool.tile(
                        [128, 4, 4, 128], F32, name=f"ps{h}", tag=("psA", "psB")[h]
                    )
                    j0 = 8 * p + 4 * h
                    for t in range(4):
                        for kx in range(3):
                            nc.tensor.matmul(
                                pst[:, t, :, :],
                                wt[:, r, 3 * t + kx, :],
                                v[:, t, j0 : j0 + 4, kx : kx + W],
                                start=(kx == 0), stop=(kx == 2),
                            )
                    if split_final:
                        # two-tap evacuations so the inverse can start while
                        # taps 2-3 are still being evacuated; yev reads tap 2
                        # straight from PSUM to shorten the chain further
                        r0_ = 16 * p + 8 * h
                        nc.scalar.copy(
                            ms[:, 0:2, 4 * h : 4 * h + 4, :], pst[:, 0:2, :, :]
                        )
                        nc.scalar.copy(
                            ms[:, 2:4, 4 * h : 4 * h + 4, :], pst[:, 2:4, :, :]
                        )
                        msh = ms[:, :, 4 * h : 4 * h + 4, :]
                        t01 = ypool.tile([128, 4, 128], BF16, name="t01h", tag="t01")
                        t12 = ypool.tile([128, 4, 128], BF16, name="t12h", tag="t12")
                        yt = ypool.tile([128, 4, 2, 128], BF16, name="yth", tag="yt")
                        nc.vector.tensor_tensor(t01[:, :, :], msh[:, 0], msh[:, 1], ALU.add)
                        nc.vector.tensor_tensor(
                            yt[:, :, 0, :], t01[:, :, :], msh[:, 2], ALU.add
                        )
                        nc.vector.tensor_tensor(t12[:, :, :], msh[:, 1], msh[:, 2], ALU.subtract)
                        nc.vector.tensor_tensor(
                            yt[:, :, 1, :], t12[:, :, :], msh[:, 3], ALU.subtract
                        )
                        b, h0 = store
                        for q in range(2):
                            rq = r0_ + 4 * q
                            ob_f = obuf[:, rq : rq + 4, :].rearrange("p r c -> p (r c)")
                            ac_f = acc[:, rq : rq + 4, :].rearrange("p r c -> p (r c)")
                            yt_f = yt[:, 2 * q : 2 * q + 2, :, :].rearrange(
                                "p j q c -> p (j q c)"
                            )
                            nc.vector.scalar_tensor_tensor(
                                ob_f, yt_f, 0.0, ac_f, ALU.bypass, ALU.max
                            )
                            nc.sync.dma_start(
                                out=y[b, :, h0 + rq : h0 + rq + 4, :],
                                in_=obuf[:, rq : rq + 4, :],
                            )
                    else:
                        nc.scalar.copy(ms[:, :, 4 * h : 4 * h + 4, :], pst[:, :, :, :])
                if split_final:
                    return

                ev = acc[:, 16 * p : 16 * p + 16 : 2, :]
                od = acc[:, 16 * p + 1 : 16 * p + 16 : 2, :]
                t01 = ypool.tile([128, 8, 128], BF16, name="t01", tag="t01")
                t12 = ypool.tile([128, 8, 128], BF16, name="t12", tag="t12")
                nc.vector.tensor_tensor(t01[:, :, :], ms[:, 0], ms[:, 1], ALU.add)
                nc.vector.tensor_tensor(t12[:, :, :], ms[:, 1], ms[:, 2], ALU.subtract)
                if r == 0:
                    nc.vector.tensor_tensor(ev, t01[:, :, :], ms[:, 2], ALU.add)
                    nc.vector.tensor_tensor(od, t12[:, :, :], ms[:, 3], ALU.subtract)
                    return
                # yt is laid out (j, parity, c) so the row-interleaved view is
                # a contiguous flatten: one max op covers all 16 rows (fewer
                # DVE ops -> fewer DRAIN bubbles)
                yt = ypool.tile([128, 8, 2, 128], BF16, name="yt", tag="yt")
                nc.vector.tensor_tensor(yt[:, :, 0, :], t01[:, :, :], ms[:, 2], ALU.add)
                nc.vector.tensor_tensor(yt[:, :, 1, :], t12[:, :, :], ms[:, 3], ALU.subtract)
                ac_f = acc[:, 16 * p : 16 * p + 16, :].rearrange("p r c -> p (r c)")
                yt_f = yt[:, :, :, :].rearrange("p j q c -> p (j q c)")
                if r < R - 1:
                    nc.vector.tensor_tensor(ac_f, ac_f, yt_f, ALU.max)
                else:
                    # final rotation: fused max + f32 upcast into obuf
                    ob_f = obuf[:, 16 * p : 16 * p + 16, :].rearrange(
                        "p r c -> p (r c)"
                    )
                    nc.vector.scalar_tensor_tensor(
                        ob_f, yt_f, 0.0, ac_f, ALU.bypass, ALU.max
                    )
                    if store is not None:
                        b, h0 = store
                        nc.sync.dma_start(
                            out=y[b, :, h0 + 16 * p : h0 + 16 * p + 16, :],
                            in_=obuf[:, 16 * p : 16 * p + 16, :],
                        )

            # DMA issue order (the sync queue issues serially): block 0's
            # first x rows (V j 0..3 needs x rows <= 9), then r=0's weights,
            # then the rest of block 0, the other rotations, block 1, ...
            xst0 = xst2[0]
            nc.sync.dma_start(out=xst0[:, 1:11, 1 : W + 1], in_=xs[0, :, 0:10, :])
            nc.sync.dma_start(out=xst0[:, 11:19, 1 : W + 1], in_=xs[0, :, 10:18, :])
            nc.sync.dma_start(out=xst0[:, 19:34, 1 : W + 1], in_=xs[0, :, 18:33, :])
            for r in range(1, R):
                nc.sync.dma_start(out=wt[:, r], in_=wl[:, r])
            v0 = vpool.tile([128, 4, JB, W + 2], BF16, name="v0", tag="v")
            emit_v(xst0, v0, jcuts=(0, 4, 8, 16))

            NB = BL * NBLK
            vcur = v0
            for g in range(NB):
                b, blk = divmod(g, NBLK)
                acc = apool.tile([128, RB, W], BF16, name="acc", tag="acc")
                obuf = opool.tile([128, RB, W], F32, name="obuf", tag="out")
                vnext = None
                last_blk = g == NB - 1
                for r in range(R):
                    for p in range(2):
                        conv_pair(
                            vcur, acc, obuf, r, p,
                            store=(b, blk * RB) if r == R - 1 else None,
                            split_final=(last_blk and r == R - 1),
                        )
                    if r == 0 and g + 1 < NB:
                        vnext = stage(g + 1)
                vcur = vnext
    nc.finalize()
    return nc


def _get_nc():
    if "nc" not in _NC_CACHE:
        _NC_CACHE["nc"] = _build()
    return _NC_CACHE["nc"]


def kernel(x, weight, rot_alpha):
    global LAST_RESULTS
    x = np.ascontiguousarray(np.asarray(x, np.float32))
    weight = np.ascontiguousarray(np.asarray(weight, np.float32))
    rot_alpha = np.asarray(rot_alpha, np.float32)

    wl = _wino_weights(weight, rot_alpha)

    nc = _get_nc()
    in_maps = [
        {"xs": np.ascontiguousarray(x[c * BL : (c + 1) * BL]), "wl": wl}
        for c in range(NCORES)
    ]
    try:
        res = run_bass_kernel_spmd(nc, in_maps, list(range(NCORES)), trace=_TRACE)
    except Exception:
        # One retry (without tracing): a failed compile or an aborted run can
        # leave a NeuronCore transiently wedged; the next attempt recovers.
        res = run_bass_kernel_spmd(nc, in_maps, list(range(NCORES)), trace=False)
    LAST_RESULTS = res
    return np.concatenate([res.results[c]["y"] for c in range(NCORES)], axis=0)


# revision 3
# speedup vs baseline: 1.1032x; 1.1032x over previous
"""Equivariant rotation conv for Trainium2, 8-core batch-parallel,
F(2,3) Winograd along H.

Computes: rotate a (128*8, 128, 3, 3) filter bank by 8 data-dependent angles
(bilinear resampling), run a 3x3 same-padded conv of x (16,128,128,128) with
all 8*128 rotated filters, then max over the 8 rotations -> (16,128,128,128).

Sharding: data-parallel over batch, 2 images per core; weights replicated.

Algorithm (per core):
  - Weight prep happens fully on host: the 9x9 bilinear rotation-mixing
    matrix is applied to each rotation's filter bank, then the Winograd
    F(2,3) G-transform is folded along ky, giving per rotation 4 H-taps x
    3 kx-taps of [cin, O] bf16 lhsT matrices (shipped, no device prep).
  - Input transform on DVE: V_t[j] over row pairs (4 taps, 1 add each,
    f32 in -> bf16 out), columns padded same as the direct kernel.
  - Per 8-output-row "pair" per rotation: 24 matmuls (4 t x 3 kx x 2
    half-groups of N=512) accumulate kx in f32 PSUM (2x 4-bank tiles,
    alternating halves).
  - ScalarE evacuates each 4-bank PSUM half (2048 f32) to SBUF bf16 --
    this keeps the DVE in its 2x bf16 mode and off the slow PSUM port.
  - DVE inverse transform: yev = (m0+m1)+m2, yod = (m1-m2)-m3, then a
    running elementwise max over rotations (skipped for r=0; the final
    rotation fuses max with the f32 upcast via scalar_tensor_tensor).
  - PE work is 2/3 of the direct 9-tap conv: ~164 matmul-cycles per
    output pixel per rotation instead of 288.
"""

import numpy as np


def _install_axon_hooks_shim():
    """Provide antenv.axon_hooks (NTFF profile hook) when the image's antenv
    lacks it, so run_bass_kernel_spmd(trace=True) works instead of crashing
    on import."""
    import contextlib
    import ctypes
    import os
    import sys
    import types

    try:
        import antenv.axon_hooks  # noqa: F401

        return
    except ImportError:
        pass

    state = {"hook": None, "resolved": False}

    def _make_hook():
        so_path = os.environ.get("AXON_PJRT_SO", "/opt/axon/libaxon_pjrt.so")
        if not os.path.exists(so_path):
            return None
        lib = ctypes.CDLL(so_path)
        if not hasattr(lib, "axon_start_nrt_profile"):
            return None
        lib.axon_start_nrt_profile.argtypes = [
            ctypes.POINTER(ctypes.c_int64),
            ctypes.c_size_t,
        ]
        lib.axon_start_nrt_profile.restype = ctypes.c_int64
        lib.axon_stop_nrt_profile.argtypes = [ctypes.c_char_p]
        lib.axon_stop_nrt_profile.restype = ctypes.c_int64

        @contextlib.contextmanager
        def _hook(output_dir, device_ids):
            import jax

            jax.devices()
            if device_ids:
                ids = (ctypes.c_int64 * len(device_ids))(*device_ids)
                rc = lib.axon_start_nrt_profile(ids, len(device_ids))
            else:
                rc = lib.axon_start_nrt_profile(None, 0)
            if rc != 0:
                raise RuntimeError(f"axon_start_nrt_profile rc={rc}")
            try:
                yield
            finally:
                n = lib.axon_stop_nrt_profile(str(output_dir).encode())
                if n < 0:
                    raise RuntimeError(f"axon_stop_nrt_profile rc={n}")
                print(f"profile: {n} file(s) written to {output_dir}")

        return _hook

    mod = types.ModuleType("antenv.axon_hooks")

    def set_axon_ntff_profile_hook(h):
        state["hook"] = h
        state["resolved"] = True

    def get_axon_ntff_profile_hook():
        if not state["resolved"]:
            state["hook"] = _make_hook()
            state["resolved"] = True
        return state["hook"]

    mod.set_axon_ntff_profile_hook = set_axon_ntff_profile_hook
    mod.get_axon_ntff_profile_hook = get_axon_ntff_profile_hook
    sys.modules["antenv.axon_hooks"] = mod


_install_axon_hooks_shim()

import ml_dtypes

import concourse.bass as bass
import concourse.mybir as mybir
from concourse import bacc
from concourse.bass_utils import run_bass_kernel_spmd
from concourse.tile import TileContext

F32 = mybir.dt.float32
BF16 = mybir.dt.bfloat16
ALU = mybir.AluOpType

B, CIN, H, W = 16, 128, 128, 128
R, O, K = 8, 128, 3
NCORES = 8
BL = B // NCORES   # images per core
RB = 32            # output rows per block
JB = RB // 2       # winograd row-pairs per block
NBLK = H // RB

_TRACE = False
LAST_RESULTS = None
_NC_CACHE = {}


def _rot_mats(rot_alpha):
    """Per-rotation 9x9 bilinear resampling matrices, matching the reference
    F.grid_sample(align_corners=True, zeros) tap logic exactly."""
    M = np.zeros((R, 9, 9), np.float64)
    lin = np.linspace(-1.0, 1.0, K)
    for r in range(R):
        ang = float(rot_alpha[r]) * (np.pi / 4.0) * r
        c, s = np.cos(ang), np.sin(ang)
        for a in range(K):          # output row (gy = lin[a])
            for b in range(K):      # output col (gx = lin[b])
                gx, gy = lin[b], lin[a]
                xs = c * gx - s * gy
                ys = s * gx + c * gy
                ix = (xs + 1.0) * 0.5 * (K - 1)
                iy = (ys + 1.0) * 0.5 * (K - 1)
                x0 = int(np.floor(ix))
                y0 = int(np.floor(iy))
                wx, wy = ix - x0, iy - y0
                p = a * K + b
                for yi, xi, wt in (
                    (y0, x0, (1 - wy) * (1 - wx)),
                    (y0, x0 + 1, (1 - wy) * wx),
                    (y0 + 1, x0, wy * (1 - wx)),
                    (y0 + 1, x0 + 1, wy * wx),
                ):
                    if 0 <= yi < K and 0 <= xi < K:
                        M[r, p, yi * K + xi] += wt
    return M


def _wino_weights(weight, rot_alpha):
    """lhsT bank [cin, R, 12, O] bf16: rotated filters with the F(2,3)
    G-transform folded along ky; 12 = (t, kx) pairs."""
    M = _rot_mats(rot_alpha)
    Bw = weight.reshape(O, R, CIN, K * K).astype(np.float64)  # [o, r, i, q]
    wr = np.einsum("rpq,oriq->roip", M, Bw).reshape(R, O, CIN, K, K)
    G = np.array(
        [[1, 0, 0], [0.5, 0.5, 0.5], [0.5, -0.5, 0.5], [0, 0, 1]], np.float64
    )
    wt = np.einsum("ty,roiyx->irtxo", G, wr)  # [i, r, t, kx, o]
    wt = wt.reshape(CIN, R, 12, O).astype(np.float32)
    return np.ascontiguousarray(wt).astype(ml_dtypes.bfloat16)


def _build():
    nc = bacc.Bacc(trn_type="TRN2")
    xs = nc.dram_tensor("xs", [BL, CIN, H, W], F32, kind="ExternalInput")
    wl = nc.dram_tensor("wl", [CIN, R, 12, O], BF16, kind="ExternalInput")
    y = nc.dram_tensor("y", [BL, O, H, W], F32, kind="ExternalOutput")

    with TileContext(nc) as tc:
        with (
            tc.tile_pool(name="wgt", bufs=1) as wpool,
            tc.tile_pool(name="xio", bufs=1) as xpool,
            tc.tile_pool(name="vst", bufs=2) as vpool,
            tc.tile_pool(name="mst", bufs=5) as mpool,
            tc.tile_pool(name="yst", bufs=2) as ypool,
            tc.tile_pool(name="accp", bufs=2) as apool,
            tc.tile_pool(name="outp", bufs=2) as opool,
            tc.tile_pool(name="psum", bufs=1, space="PSUM") as ppool,
        ):
            wt = wpool.tile([128, R, 12, O], BF16, name="wt", tag="wt")

            # PE warm-up: dependency-free matmuls keep the PE busy from
            # ~0.5us so the HAM clock gate reaches 8/8 before real work
            # starts at ~6us.
            # dum_lhs zeroing must be the vector engine's first op: ACT pays
            # a ~2.7us activation-table load before its first instruction,
            # and gpsimd a ~6us ucode load -- either would delay the warm-up.
            dum_lhs = wpool.tile([128, 128], BF16, name="dum_lhs", tag="dum")
            nc.vector.memset(dum_lhs[:, :], 0.0)
            # r=0 weights issued from the scalar engine's DMA queue (its
            # first instruction): transfers in parallel with block 0's first
            # x rows on the sync queue
            nc.scalar.dma_start(out=wt[:, 0], in_=wl[:, 0])
            dum_ps = ppool.tile([128, 4, 4, 128], F32, name="dum_ps", tag="psA")
            # enough dummies to bridge all the way to the first real matmul
            # (~13-14us): a PE-idle gap there crosses the HAM MID window and
            # re-throttles the clock to 1.2GHz for the first real microseconds
            for _ in range(72):
                nc.tensor.matmul(
                    dum_ps[:, 0, 0, :], dum_lhs[:, :], dum_lhs[:, :],
                    start=True, stop=True,
                )

            # x staging: manual ping-pong between two persistent buffers so
            # the zero padding (columns 0 and W+1, boundary halo rows) is
            # established once instead of re-memset every block.
            # Only the pad regions need zeroing (DMA always writes cols
            # 1..W, so cols 0 and W+1 are established once; halo rows are
            # re-zeroed by load_x when a buffer is reused at an image edge).
            # Vector-engine memsets (fast, and the scalar queue must stay
            # clear so the r=0 weights DMA issues immediately).
            xst2 = [
                xpool.tile([128, RB + 2, W + 2], F32, name=f"xst{i}", tag=f"xst{i}")
                for i in range(2)
            ]
            for i in range(2):
                nc.vector.memset(xst2[i][:, :, 0:1], 0.0)
                nc.vector.memset(xst2[i][:, :, W + 1 : W + 2], 0.0)
                nc.vector.memset(xst2[i][:, 0:1, 1 : W + 1], 0.0)
                nc.vector.memset(xst2[i][:, RB + 1 : RB + 2, 1 : W + 1], 0.0)

            def load_x(g, b, blk, cuts=None):
                h0 = blk * RB
                r0 = max(h0 - 1, 0)
                r1 = min(h0 + RB + 1, H)
                xst = xst2[g % 2]
                if g >= 2:
                    # restore halo-row zeros clobbered by the previous user
                    # of this buffer (interior blocks write all 34 rows)
                    if blk == 0:
                        nc.gpsimd.memset(xst[:, 0:1, :], 0.0)
                    elif blk == NBLK - 1:
                        nc.gpsimd.memset(xst[:, RB + 1 : RB + 2, :], 0.0)
                d0 = r0 - (h0 - 1)
                nrows = r1 - r0
                if cuts is None:
                    cuts = [0, nrows]
                for k in range(len(cuts) - 1):
                    a, c = cuts[k], cuts[k + 1]
                    nc.sync.dma_start(
                        out=xst[:, d0 + a : d0 + c, 1 : W + 1],
                        in_=xs[b, :, r0 + a : r0 + c, :],
                    )
                return xst

            # V input transform: V0=x[2j-1]-x[2j+1], V1=x[2j]+x[2j+1],
            # V2=x[2j+1]-x[2j], V3=x[2j]-x[2j+3+...]; xst row d holds
            # x row (h0-1)+d so x[2j+k] is xst row 2jl+k+1.
            VSPEC = [
                (0, 2, ALU.subtract),
                (1, 2, ALU.add),
                (2, 1, ALU.subtract),
                (1, 3, ALU.subtract),
            ]

            def emit_v(xst, v, jcuts=(0, JB)):
                for k in range(len(jcuts) - 1):
                    j0, j1 = jcuts[k], jcuts[k + 1]
                    for t, (a, bb, op) in enumerate(VSPEC):
                        nc.vector.tensor_tensor(
                            v[:, t, j0:j1, :],
                            xst[:, a + 2 * j0 : a + 2 * j1 - 1 : 2, :],
                            xst[:, bb + 2 * j0 : bb + 2 * j1 - 1 : 2, :],
                            op,
                        )

            def stage(g, cuts=None, jcuts=(0, JB)):
                b, blk = divmod(g, NBLK)
                xst = load_x(g, b, blk, cuts=cuts)
                v = vpool.tile([128, 4, JB, W + 2], BF16, name=f"v{g}", tag="v")
                emit_v(xst, v, jcuts)
                return v

            def conv_pair(v, acc, obuf, r, p, store=None, split_final=False):
                # One pair = 8 output rows (j = 8p .. 8p+8).  24 matmuls in
                # two 4-bank halves; ScalarE evacuates each half to bf16;
                # DVE does the 4-add inverse + running max.  split_final
                # (the very last pair) pipelines everything per half so the
                # post-matmul tail is short.
                ms = mpool.tile([128, 4, 8, 128], BF16, name="ms", tag="ms")
                for h in range(2):
                    pst = ppool.tile(
                        [128, 4, 4, 128], F32, name=f"ps{h}", tag=("psA", "psB")[h]
                    )
                    j0 = 8 * p + 4 * h
                    for t in range(4):
                        for kx in range(3):
                            nc.tensor.matmul(
                                pst[:, t, :, :],
                                wt[:, r, 3 * t + kx, :],
                                v[:, t, j0 : j0 + 4, kx : kx + W],
                                start=(kx == 0), stop=(kx == 2),
                            )
                    if split_final:
                        # two-tap evacuations so the inverse can start while
                        # taps 2-3 are still being evacuated; yev reads tap 2
                        # straight from PSUM to shorten the chain further
                        r0_ = 16 * p + 8 * h
                        nc.scalar.copy(
                            ms[:, 0:2, 4 * h : 4 * h + 4, :], pst[:, 0:2, :, :]
                        )
                        nc.scalar.copy(
                            ms[:, 2:4, 4 * h : 4 * h + 4, :], pst[:, 2:4, :, :]
                        )
                        msh = ms[:, :, 4 * h : 4 * h + 4, :]
                        t01 = ypool.tile([128, 4, 128], BF16, name="t01h", tag="t01")
                        t12 = ypool.tile([128, 4, 128], BF16, name="t12h", tag="t12")
                        yt = ypool.tile([128, 4, 2, 128], BF16, name="yth", tag="yt")
                        nc.vector.tensor_tensor(t01[:, :, :], msh[:, 0], msh[:, 1], ALU.add)
                        nc.vector.tensor_tensor(
                            yt[:, :, 0, :], t01[:, :, :], msh[:, 2], ALU.add
                        )
                        nc.vector.tensor_tensor(t12[:, :, :], msh[:, 1], msh[:, 2], ALU.subtract)
                        nc.vector.tensor_tensor(
                            yt[:, :, 1, :], t12[:, :, :], msh[:, 3], ALU.subtract
                        )
                        b, h0 = store
                        for q in range(2):
                            rq = r0_ + 4 * q
                            ob_f = obuf[:, rq : rq + 4, :].rearrange("p r c -> p (r c)")
                            ac_f = acc[:, rq : rq + 4, :].rearrange("p r c -> p (r c)")
                            yt_f = yt[:, 2 * q : 2 * q + 2, :, :].rearrange(
                                "p j q c -> p (j q c)"
                            )
                            nc.vector.scalar_tensor_tensor(
                                ob_f, yt_f, 0.0, ac_f, ALU.bypass, ALU.max
                            )
                            nc.sync.dma_start(
                                out=y[b, :, h0 + rq : h0 + rq + 4, :],
                                in_=obuf[:, rq : rq + 4, :],
                            )
                    else:
                        nc.scalar.copy(ms[:, :, 4 * h : 4 * h + 4, :], pst[:, :, :, :])
                if split_final:
                    return

                ev = acc[:, 16 * p : 16 * p + 16 : 2, :]
                od = acc[:, 16 * p + 1 : 16 * p + 16 : 2, :]
                t01 = ypool.tile([128, 8, 128], BF16, name="t01", tag="t01")
                t12 = ypool.tile([128, 8, 128], BF16, name="t12", tag="t12")
                nc.vector.tensor_tensor(t01[:, :, :], ms[:, 0], ms[:, 1], ALU.add)
                nc.vector.tensor_tensor(t12[:, :, :], ms[:, 1], ms[:, 2], ALU.subtract)
                if r == 0:
                    nc.vector.tensor_tensor(ev, t01[:, :, :], ms[:, 2], ALU.add)
                    nc.vector.tensor_tensor(od, t12[:, :, :], ms[:, 3], ALU.subtract)
                    return
                # yt is laid out (j, parity, c) so the row-interleaved view is
                # a contiguous flatten: one max op covers all 16 rows (fewer
                # DVE ops -> fewer DRAIN bubbles)
                yt = ypool.tile([128, 8, 2, 128], BF16, name="yt", tag="yt")
                nc.vector.tensor_tensor(yt[:, :, 0, :], t01[:, :, :], ms[:, 2], ALU.add)
                nc.vector.tensor_tensor(yt[:, :, 1, :], t12[:, :, :], ms[:, 3], ALU.subtract)
                ac_f = acc[:, 16 * p : 16 * p + 16, :].rearrange("p r c -> p (r c)")
                yt_f = yt[:, :, :, :].rearrange("p j q c -> p (j q c)")
                if r < R - 1:
                    nc.vector.tensor_tensor(ac_f, ac_f, yt_f, ALU.max)
                else:
                    # final rotation: fused max + f32 upcast into obuf
                    ob_f = obuf[:, 16 * p : 16 * p + 16, :].rearrange(
                        "p r c -> p (r c)"
                    )
                    nc.vector.scalar_tensor_tensor(
                        ob_f, yt_f, 0.0, ac_f, ALU.bypass, ALU.max
                    )
                    if store is not None:
                        b, h0 = store
                        nc.sync.dma_start(
                            out=y[b, :, h0 + 16 * p : h0 + 16 * p + 16, :],
                            in_=obuf[:, 16 * p : 16 * p + 16, :],
                        )

            # DMA issue order (the sync queue issues serially): block 0's
            # first x rows (V j 0..3 needs x rows <= 9), then r=0's weights,
            # then the rest of block 0, the other rotations, block 1, ...
            xst0 = xst2[0]
            nc.sync.dma_start(out=xst0[:, 1:7, 1 : W + 1], in_=xs[0, :, 0:6, :])
            nc.sync.dma_start(out=xst0[:, 7:11, 1 : W + 1], in_=xs[0, :, 6:10, :])
            nc.sync.dma_start(out=xst0[:, 11:19, 1 : W + 1], in_=xs[0, :, 10:18, :])
            nc.sync.dma_start(out=xst0[:, 19:34, 1 : W + 1], in_=xs[0, :, 18:33, :])
            for r in range(1, R):
                nc.sync.dma_start(out=wt[:, r], in_=wl[:, r])
            v0 = vpool.tile([128, 4, JB, W + 2], BF16, name="v0", tag="v")
            emit_v(xst0, v0, jcuts=(0, 2, 4, 8, 16))

            NB = BL * NBLK
            vcur = v0
            for g in range(NB):
                b, blk = divmod(g, NBLK)
                acc = apool.tile([128, RB, W], BF16, name="acc", tag="acc")
                obuf = opool.tile([128, RB, W], F32, name="obuf", tag="out")
                vnext = None
                last_blk = g == NB - 1
                for r in range(R):
                    for p in range(2):
                        conv_pair(
                            vcur, acc, obuf, r, p,
                            store=(b, blk * RB) if r == R - 1 else None,
                            split_final=(last_blk and r == R - 1),
                        )
                    if r == 0 and g + 1 < NB:
                        vnext = stage(g + 1)
                vcur = vnext
    nc.finalize()
    return nc


def _get_nc():
    if "nc" not in _NC_CACHE:
        _NC_CACHE["nc"] = _build()
    return _NC_CACHE["nc"]


def kernel(x, weight, rot_alpha):
    global LAST_RESULTS
    x = np.ascontiguousarray(np.asarray(x, np.float32))
    weight = np.ascontiguousarray(np.asarray(weight, np.float32))
    rot_alpha = np.asarray(rot_alpha, np.float32)

    wl = _wino_weights(weight, rot_alpha)

    nc = _get_nc()
    in_maps = [
        {"xs": np.ascontiguousarray(x[c * BL : (c + 1) * BL]), "wl": wl}
        for c in range(NCORES)
    ]
    try:
        res = run_bass_kernel_spmd(nc, in_maps, list(range(NCORES)), trace=_TRACE)
    except Exception:
        # One retry (without tracing): a failed compile or an aborted run can
        # leave a NeuronCore transiently wedged; the next attempt recovers.
        res = run_bass_kernel_spmd(nc, in_maps, list(range(NCORES)), trace=False)
    LAST_RESULTS = res
    return np.concatenate([res.results[c]["y"] for c in range(NCORES)], axis=0)
